# revision 2
# baseline (speedup 1.0000x reference)
"""GCN graph-embedding kernel v2 for 8 Trainium2 NeuronCores (Bass/Tile).

Strategy (dst-node sharding, dma_gather edition):
  - Nodes permuted + bin-packed into 128-node blocks balanced by in-degree,
    49 blocks/core. Per-block regular-edge lists are split into two int16
    index windows (lo = table rows [0, 32768), hi = rows [vpad-32768, vpad)),
    sorted by source row, and packed column-major into 128-edge chunks.
  - Source rows are fetched with InstDMAGatherAnt (SWDGE Q7 gather):
    1024 indices per instruction (HW limit), bf16 256B rows. The per-
    instruction descriptor-gen cost is ~1.5us + 0.34ns/row, ~6x cheaper per
    edge than per-128-row indirect DMAs.
  - Normalization is folded into the data: the gather table holds
    x~ = dinv[n] * x[n] (and between layers h~1 = dinv * relu(...)), so the
    aggregation is a plain masked sum; the remaining dinv[dst] factor is a
    per-partition activation scale after the W matmul.
  - Per chunk, a selector matrix sel[e, slot] = (iota == dstrel) routes edges
    to their dst slot on the TensorEngine, accumulating agg^T in PSUM.
    Selectors for a whole gather instruction are built in one DVE op via a
    stride-0 broadcast of the dstrel columns.
  - Self-loops skip the gather: the core's own (prescaled) block arrives via
    one strided DMA and joins the PSUM accumulation as an identity matmul.
  - Between layers each core's h~1 slice is exchanged with a chunked
    AllGather (chunk-major h table, pid2 addressing) exactly as in v1.
  - Global mean-pool is fused into layer 2 as a one-hot matmul accumulated in
    PSUM; partials are combined with a small AllReduce and the tiny linear
    head runs redundantly on every core.
"""
import numpy as np

import concourse.bass as bass
import concourse.bacc as bacc
import concourse.mybir as mybir
import concourse.tile as tile
from concourse.bass_utils import run_bass_kernel_spmd

F = 128          # feature width (all layers)
P = 128          # partitions / block size
CORES = 8
BPC = 49         # blocks per core
NG = 64          # number of graphs
WIN = 32768      # int16 index window size
CPI = 8          # max chunks per gather instruction (1024 idx HW cap)


def split_multi_waits(nc, max_waits: int = 1) -> int:
    n_split = 0
    f = nc.cur_f
    for bb in f.blocks:
        new_insts = []
        for inst in bb.instructions:
            si = inst.sync_info
            if si is not None and len(si.on_wait) > max_waits:
                waits = list(si.on_wait)
                extra, keep = waits[:-max_waits], waits[-max_waits:]
                for w in extra:
                    nop = mybir.InstNoOp(
                        name=nc.get_next_instruction_name(),
                        sync_info=mybir.SyncInfo(on_wait=[w], on_update=[]),
                        bass_nofuse=True,
                        engine=inst.engine,
                        ins=[],
                        outs=[],
                    )
                    nc.register_instruction(nop, overwrite=True)
                    new_insts.append(nop)
                inst.sync_info = mybir.SyncInfo(
                    on_wait=keep, on_update=list(si.on_update)
                )
                n_split += 1
            new_insts.append(inst)
        bb.instructions = new_insts
    return n_split


def _pack_layer(src_idx, e_dst_block, e_dst_slot, nblocks, n_cores, bpc):
    """Pack one layer's regular edges into lo/hi 128-edge chunk grids.

    src_idx: per-edge source row in the layer's gather table (pid or pid2).
    Returns per-core idx16 tables, dstrel tables, instruction metas and
    chunk-location maps (shared across cores: the shapes are uniform).
    """
    vpad = nblocks * P
    hi_base = vpad - WIN

    eorder = np.argsort(e_dst_block, kind="stable")
    es_idx = src_idx[eorder]
    es_slot = e_dst_slot[eorder]
    eb = np.bincount(e_dst_block, minlength=nblocks)
    eb_cum = np.concatenate([[0], np.cumsum(eb)])

    K = int(np.ceil(eb.max() / P))
    K_lo = (K + 1) // 2
    K_hi = K - K_lo
    cap_lo, cap_hi = K_lo * P, K_hi * P

    # per-block lo/hi assignment (balanced via the overlap window)
    blk_lo = []   # per block: (idx_local int16 [cap_lo], dstrel [cap_lo])
    blk_hi = []
    for b in range(nblocks):
        bi = es_idx[eb_cum[b]:eb_cum[b + 1]]
        bs = es_slot[eb_cum[b]:eb_cum[b + 1]]
        n = len(bi)
        forced_lo = bi < hi_base
        forced_hi = bi >= WIN
        mid = ~forced_lo & ~forced_hi
        n_flo, n_mid = int(forced_lo.sum()), int(mid.sum())
        assert n_flo <= cap_lo and int(forced_hi.sum()) <= cap_hi, (
            b, n_flo, int(forced_hi.sum()))
        # fill lo toward n/2, clamped by capacity on both sides
        want_lo = max(n_flo, min(n_flo + n_mid, (n + 1) // 2, cap_lo, n - 0))
        if n - want_lo > cap_hi:
            want_lo = n - cap_hi
        assert n_flo <= want_lo <= n_flo + n_mid and want_lo <= cap_lo
        take_mid = want_lo - n_flo
        lo_mask = forced_lo.copy()
        mid_pos = np.where(mid)[0]
        lo_mask[mid_pos[:take_mid]] = True
        for mask, cap, base, out in (
            (lo_mask, cap_lo, 0, blk_lo),
            (~lo_mask, cap_hi, hi_base, blk_hi),
        ):
            ii = bi[mask]
            ss = bs[mask]
            o = np.argsort(ii, kind="stable")
            ii, ss = ii[o] - base, ss[o]
            m = len(ii)
            assert m <= cap
            idx = np.zeros(cap, dtype=np.int16)
            rel = np.full(cap, -1.0, dtype=np.float32)
            idx[:m] = ii.astype(np.int16)
            rel[:m] = ss.astype(np.float32)
            out.append((idx, rel))

    # per-core chunk streams + instruction packing
    n_lo_chunks = bpc * K_lo
    n_hi_chunks = bpc * K_hi
    lo_n_instr = int(np.ceil(n_lo_chunks / CPI))
    hi_n_instr = int(np.ceil(n_hi_chunks / CPI))

    # issue order: interleave lo/hi instructions
    issue = []   # (window, start_chunk, n_chunks)
    li = hi = 0
    while li < lo_n_instr or hi < hi_n_instr:
        if li < lo_n_instr:
            s = li * CPI
            issue.append(("lo", s, min(CPI, n_lo_chunks - s)))
            li += 1
        if hi < hi_n_instr:
            s = hi * CPI
            issue.append(("hi", s, min(CPI, n_hi_chunks - s)))
            hi += 1

    # global column position of each instruction's first chunk
    instr_col0 = np.cumsum([0] + [m[2] for m in issue])[:-1]
    total_chunks = n_lo_chunks + n_hi_chunks

    # map (window, global window-chunk id) -> (instr index, pos)
    lo_map = {}
    hi_map = {}
    for i, (win, s, nch) in enumerate(issue):
        for p_ in range(nch):
            (lo_map if win == "lo" else hi_map)[s + p_] = (i, p_)

    # chunk_loc[b] = ordered (instr, pos) covering block b's K chunks
    chunk_loc = []
    need_instr = []
    for lb in range(bpc):
        locs = [lo_map[lb * K_lo + k] for k in range(K_lo)]
        locs += [hi_map[lb * K_hi + k] for k in range(K_hi)]
        chunk_loc.append(locs)
        need_instr.append(max(i for i, _ in locs))

    # per-core tables in issue order
    idx16 = np.zeros((n_cores, P, total_chunks * CPI), dtype=np.int16)
    dstrel = np.full((n_cores, P, total_chunks), -1.0, dtype=np.float32)
    for c in range(n_cores):
        col = 0
        for win, s, nch in issue:
            blk = blk_lo if win == "lo" else blk_hi
            kk = K_lo if win == "lo" else K_hi
            for p_ in range(nch):
                wc = s + p_        # window chunk id
                lb, k = divmod(wc, kk)
                idx, rel = blk[c * bpc + lb]
                ch_idx = idx[k * P:(k + 1) * P]
                ch_rel = rel[k * P:(k + 1) * P]
                # interleaved idx layout: idx j at (j%16, col*8 + j//16)
                lay = ch_idx.reshape(CPI, 16).T     # [16, 8]
                for g in range(CPI):
                    idx16[c, g * 16:(g + 1) * 16,
                          (col + p_) * CPI:(col + p_ + 1) * CPI] = lay
                dstrel[c, :, col + p_] = ch_rel
            col += nch

    meta = dict(issue=issue, instr_col0=instr_col0.tolist(),
                chunk_loc=chunk_loc, need_instr=need_instr,
                total_chunks=total_chunks, K=K)
    return idx16, dstrel, meta


def _prep(x, edge_index, batch, n_cores=CORES, bpc=BPC, ng=NG):
    """Host-side preprocessing: node permutation, degree norms, bf16 table,
    per-layer gather-chunk packing. Index manipulation + one prescale pass."""
    import heapq
    import ml_dtypes

    n = x.shape[0]
    src = np.asarray(edge_index[0], dtype=np.int64)
    dst = np.asarray(edge_index[1], dtype=np.int64)
    w_reg = np.bincount(dst, minlength=n).astype(np.int64)
    deg = w_reg + 1  # incl self-loop (PyG GCNConv norm)
    dinv = 1.0 / np.sqrt(deg.astype(np.float64))

    nblocks = n_cores * bpc
    cap = np.full(nblocks, P, dtype=np.int64)
    assert cap.sum() >= n

    order = np.argsort(-w_reg, kind="stable")
    heap = [(0, b) for b in range(nblocks)]
    heapq.heapify(heap)
    fill = np.zeros(nblocks, dtype=np.int64)
    node_block = np.empty(n, dtype=np.int64)
    node_slot = np.empty(n, dtype=np.int64)
    for nd in order:
        while True:
            load, b = heapq.heappop(heap)
            if fill[b] < cap[b]:
                break
        node_block[nd] = b
        node_slot[nd] = fill[b]
        fill[b] += 1
        if fill[b] < cap[b]:
            heapq.heappush(heap, (load + int(w_reg[nd]), b))

    vpad = nblocks * P
    pid = node_block * P + node_slot

    # chunk-major h-table layout for the chunked AllGather (layer 2)
    nchunks = min(4, bpc)
    last = max(1, bpc // 16)
    rest = bpc - last
    bounds = [round(i * rest / (nchunks - 1)) for i in range(nchunks)] + [bpc]
    gstart = [n_cores * P * b for b in bounds]
    lb_all = node_block % bpc
    c_all = node_block // bpc
    ch_all = np.searchsorted(bounds, lb_all, side="right") - 1
    rows_ch = np.array([(bounds[j + 1] - bounds[j]) * P for j in range(nchunks)])
    pid2 = (np.array(gstart)[ch_all] + c_all * rows_ch[ch_all]
            + (lb_all - np.array(bounds)[ch_all]) * P + node_slot)

    # prescaled bf16 gather table for layer 1
    xt = np.zeros((vpad, F), dtype=ml_dtypes.bfloat16)
    xt[pid] = (np.asarray(x, dtype=np.float64)
               * dinv[:, None]).astype(ml_dtypes.bfloat16)

    # per-layer edge packing (regular edges only; self-loops via identity)
    e_dst_block = node_block[dst]
    e_dst_slot = node_slot[dst]
    idx16_1, dstrel_1, meta1 = _pack_layer(
        pid[src], e_dst_block, e_dst_slot, nblocks, n_cores, bpc)
    idx16_2, dstrel_2, meta2 = _pack_layer(
        pid2[src], e_dst_block, e_dst_slot, nblocks, n_cores, bpc)

    # per-core per-slot tables
    dinvb = np.ones((n_cores, P, bpc), dtype=np.float32)
    batchp = np.full((n_cores, P, bpc), -1.0, dtype=np.float32)
    bt = np.asarray(batch, dtype=np.int64)
    for c in range(n_cores):
        mask = (node_block >= c * bpc) & (node_block < (c + 1) * bpc)
        sl = node_slot[mask]
        nb = node_block[mask] - c * bpc
        dinvb[c, sl, nb] = dinv[mask].astype(np.float32)
        batchp[c, sl, nb] = bt[mask].astype(np.float32)

    cnt = np.bincount(bt, minlength=ng).astype(np.float32)[:, None]
    return dict(xt=xt, idx16_1=idx16_1, dstrel_1=dstrel_1, meta1=meta1,
                idx16_2=idx16_2, dstrel_2=dstrel_2, meta2=meta2,
                dinvb=dinvb, batchp=batchp, cnt=cnt,
                vpad=vpad, bounds=bounds)


def _build(meta1, meta2, vpad, bounds, n_cores=CORES, bpc=BPC, ng=NG,
           debug=False):
    f32 = mybir.dt.float32
    bf16 = mybir.dt.bfloat16
    AF = mybir.ActivationFunctionType
    nc = bacc.Bacc(None, target_bir_lowering=False, debug=debug)

    tc1 = meta1["total_chunks"]
    tc2 = meta2["total_chunks"]
    hi_base = vpad - WIN
    slice_rows = bpc * P

    xt_p = nc.declare_dram_parameter("xt", [vpad, F], bf16, isOutput=False)
    xtown_p = nc.declare_dram_parameter("xt_own", [bpc * P, F], bf16,
                                        isOutput=False)
    idx1_p = nc.declare_dram_parameter("idx1", [P, tc1 * CPI], mybir.dt.int16,
                                       isOutput=False)
    idx2_p = nc.declare_dram_parameter("idx2", [P, tc2 * CPI], mybir.dt.int16,
                                       isOutput=False)
    dr1_p = nc.declare_dram_parameter("dr1", [P, tc1], bf16, isOutput=False)
    dr2_p = nc.declare_dram_parameter("dr2", [P, tc2], bf16, isOutput=False)
    dinvb_p = nc.declare_dram_parameter("dinvb", [P, bpc], f32, isOutput=False)
    batch_p = nc.declare_dram_parameter("batchp", [P, bpc], f32, isOutput=False)
    cnt_p = nc.declare_dram_parameter("cnt", [ng, 1], f32, isOutput=False)
    iota8_p = nc.declare_dram_parameter("iota8", [P, CPI * P], bf16,
                                        isOutput=False)
    ident_p = nc.declare_dram_parameter("ident", [P, P], bf16, isOutput=False)
    iotang_p = nc.declare_dram_parameter("iotang", [P, ng], f32, isOutput=False)
    w1_p = nc.declare_dram_parameter("W1", [F, F], bf16, isOutput=False)
    w2_p = nc.declare_dram_parameter("W2", [F, F], bf16, isOutput=False)
    wl_p = nc.declare_dram_parameter("Wl", [F, F], f32, isOutput=False)
    b1_p = nc.declare_dram_parameter("b1bc", [P, F], f32, isOutput=False)
    b2_p = nc.declare_dram_parameter("b2bc", [P, F], f32, isOutput=False)
    bl_p = nc.declare_dram_parameter("blbc", [ng, F], f32, isOutput=False)
    out_p = nc.declare_dram_parameter("out", [ng, F], f32, isOutput=True)

    with tile.TileContext(nc) as tc:
        with (
            tc.tile_pool(name="dram", bufs=1, space="DRAM") as dram,
            tc.tile_pool(name="const", bufs=1) as cp,
            tc.tile_pool(name="gp", bufs=6) as gp,
            tc.tile_pool(name="sp", bufs=6) as spool,
            tc.tile_pool(name="bp", bufs=4) as bpool,
            tc.tile_pool(name="ps", bufs=2, space="PSUM") as psp,
            tc.tile_pool(name="psagg", bufs=3, space="PSUM") as psagg,
            tc.tile_pool(name="psacc", bufs=1, space="PSUM") as psacc,
        ):
            ag_in = dram.tile([slice_rows, F], bf16)
            h_tab = dram.tile([vpad, F], bf16)
            ar_in = dram.tile([F, ng], f32)
            ar_out = dram.tile([F, ng], f32)

            idx1_sb = cp.tile([P, tc1 * CPI], mybir.dt.int16)
            nc.sync.dma_start(out=idx1_sb[:], in_=idx1_p[:])
            idx2_sb = cp.tile([P, tc2 * CPI], mybir.dt.int16)
            nc.sync.dma_start(out=idx2_sb[:], in_=idx2_p[:])
            dr1_sb = cp.tile([P, tc1], bf16)
            nc.sync.dma_start(out=dr1_sb[:], in_=dr1_p[:])
            dr2_sb = cp.tile([P, tc2], bf16)
            nc.sync.dma_start(out=dr2_sb[:], in_=dr2_p[:])
            dinvb_sb = cp.tile([P, bpc], f32)
            nc.sync.dma_start(out=dinvb_sb[:], in_=dinvb_p[:])
            batch_sb = cp.tile([P, bpc], f32)
            nc.sync.dma_start(out=batch_sb[:], in_=batch_p[:])
            iota8_sb = cp.tile([P, CPI * P], bf16)
            nc.sync.dma_start(out=iota8_sb[:], in_=iota8_p[:])
            ident_sb = cp.tile([P, P], bf16)
            nc.sync.dma_start(out=ident_sb[:], in_=ident_p[:])
            iotang_sb = cp.tile([P, ng], f32)
            nc.sync.dma_start(out=iotang_sb[:], in_=iotang_p[:])
            w1_sb = cp.tile([F, F], bf16)
            nc.sync.dma_start(out=w1_sb[:], in_=w1_p[:])
            w2_sb = cp.tile([F, F], bf16)
            nc.sync.dma_start(out=w2_sb[:], in_=w2_p[:])
            wl_sb = cp.tile([F, F], f32)
            nc.sync.dma_start(out=wl_sb[:], in_=wl_p[:])
            b1_sb = cp.tile([P, F], f32)
            nc.sync.dma_start(out=b1_sb[:], in_=b1_p[:])
            b2_sb = cp.tile([P, F], f32)
            nc.sync.dma_start(out=b2_sb[:], in_=b2_p[:])
            bl_sb = cp.tile([ng, F], f32)
            nc.sync.dma_start(out=bl_sb[:], in_=bl_p[:])
            cnt_sb = cp.tile([ng, 1], f32)
            nc.sync.dma_start(out=cnt_sb[:], in_=cnt_p[:])

            pool_acc = psacc.tile([F, ng], f32)

            def layer(meta, src_tab, self_src, idx_sb, dr_sb, w_sb, bbc_sb,
                      is_last, post_block=None):
                issue = meta["issue"]
                col0s = meta["instr_col0"]
                chunk_loc = meta["chunk_loc"]
                need_instr = meta["need_instr"]
                K = meta["K"]

                selfb = cp.tile([P, bpc * F], bf16, tag="selfb")
                nc.sync.dma_start(
                    out=selfb[:].rearrange("p (b f) -> p b f", f=F),
                    in_=self_src.rearrange("(b p) f -> p b f", p=P),
                )
                gtiles = [None] * len(issue)
                seltiles = [None] * len(issue)

                def emit_gather(i):
                    win, _s, nch = issue[i]
                    c0 = col0s[i]
                    src_win = (src_tab[0:WIN, :] if win == "lo"
                               else src_tab[hi_base:vpad, :])
                    gt = gp.tile([P, nch * F], bf16, tag="g")
                    nc.gpsimd.dma_gather(
                        gt[:].rearrange("p (c f) -> p c f", f=F),
                        src_win,
                        idx_sb[:, c0 * CPI:(c0 + nch) * CPI],
                        nch * P,
                        nch * P,
                        F,
                    )
                    st = spool.tile([P, nch * P], bf16, tag="sel")
                    nc.vector.tensor_tensor(
                        out=st[:],
                        in0=iota8_sb[:, :nch * P],
                        in1=dr_sb[:, c0:c0 + nch].to_broadcast([P, nch, P]),
                        op=mybir.AluOpType.is_equal,
                    )
                    gtiles[i] = gt
                    seltiles[i] = st

                emitted = 0
                for b in range(bpc):
                    while emitted <= need_instr[b]:
                        emit_gather(emitted)
                        emitted += 1
                    psum_agg = psagg.tile([F, P], f32, tag="agg")
                    # self-loop contribution: x~[d] via identity routing
                    nc.tensor.matmul(
                        out=psum_agg[:], lhsT=selfb[:, b * F:(b + 1) * F],
                        rhs=ident_sb[:], start=True, stop=False,
                    )
                    for j, (i, pos) in enumerate(chunk_loc[b]):
                        nc.tensor.matmul(
                            out=psum_agg[:],
                            lhsT=gtiles[i][:, pos * F:(pos + 1) * F],
                            rhs=seltiles[i][:, pos * P:(pos + 1) * P],
                            start=False, stop=(j == K - 1),
                        )
                    aggT = bpool.tile([F, P], bf16, tag="aggT")
                    nc.vector.tensor_copy(out=aggT[:], in_=psum_agg[:])
                    psum_h = psp.tile([P, F], f32, tag="h")
                    nc.tensor.matmul(out=psum_h[:], lhsT=aggT[:], rhs=w_sb[:],
                                     start=True, stop=True)
                    t1 = bpool.tile([P, F], f32, tag="t1")
                    nc.scalar.activation(out=t1[:], in_=psum_h[:], func=AF.Copy,
                                         scale=dinvb_sb[:, b:b + 1])
                    hb = bpool.tile([P, F], f32, tag="hb")
                    nc.vector.tensor_add(out=hb[:], in0=t1[:], in1=bbc_sb[:])
                    if not is_last:
                        # h~1 = dinv * relu(hb) = relu(dinv * hb); dinv > 0
                        hrt = bpool.tile([P, F], bf16, tag="hrt")
                        nc.scalar.activation(out=hrt[:], in_=hb[:], func=AF.Relu,
                                             scale=dinvb_sb[:, b:b + 1])
                        nc.sync.dma_start(
                            out=ag_in[b * P:(b + 1) * P, :], in_=hrt[:])
                    else:
                        hr = bpool.tile([P, F], bf16, tag="hr")
                        nc.scalar.activation(out=hr[:], in_=hb[:], func=AF.Relu)
                        gbh = bpool.tile([P, ng], bf16, tag="Gh")
                        nc.vector.tensor_tensor(
                            out=gbh[:],
                            in0=batch_sb[:, b:b + 1].to_broadcast([P, ng]),
                            in1=iotang_sb[:],
                            op=mybir.AluOpType.is_equal,
                        )
                        nc.tensor.matmul(out=pool_acc[:], lhsT=hr[:],
                                         rhs=gbh[:],
                                         start=(b == 0), stop=(b == bpc - 1))
                    if post_block is not None:
                        post_block(b)

            nchunks = len(bounds) - 1

            def post_block(b):
                for j in range(nchunks):
                    if b == bounds[j + 1] - 1:
                        rows = (bounds[j + 1] - bounds[j]) * P
                        gs = n_cores * P * bounds[j]
                        nc.gpsimd.collective_compute(
                            "AllGather",
                            mybir.AluOpType.bypass,
                            replica_groups=[list(range(n_cores))],
                            ins=[ag_in[bounds[j] * P:bounds[j + 1] * P, :]],
                            outs=[h_tab[gs:gs + n_cores * rows, :]],
                        )

            layer(meta1, xt_p, xtown_p[:], idx1_sb, dr1_sb, w1_sb, b1_sb,
                  is_last=False, post_block=post_block)
            layer(meta2, h_tab, ag_in[:], idx2_sb, dr2_sb, w2_sb, b2_sb,
                  is_last=True)

            poolT_sb = cp.tile([F, ng], f32)
            nc.vector.tensor_copy(out=poolT_sb[:], in_=pool_acc[:])
            nc.gpsimd.dma_start(out=ar_in[:], in_=poolT_sb[:])
            nc.gpsimd.collective_compute(
                "AllReduce",
                mybir.AluOpType.add,
                replica_groups=[list(range(n_cores))],
                ins=[ar_in.opt()],
                outs=[ar_out.opt()],
            )
            poolT_ar = cp.tile([F, ng], f32)
            nc.gpsimd.dma_start(out=poolT_ar[:], in_=ar_out[:])

            psum_o = psp.tile([ng, F], f32, tag="o")
            nc.tensor.matmul(out=psum_o[:], lhsT=poolT_ar[:], rhs=wl_sb[:],
                             start=True, stop=True)
            cmax = cp.tile([ng, 1], f32)
            nc.vector.tensor_scalar(out=cmax[:], in0=cnt_sb[:], scalar1=1.0,
                                    scalar2=None, op0=mybir.AluOpType.max)
            rcnt = cp.tile([ng, 1], f32)
            nc.vector.reciprocal(out=rcnt[:], in_=cmax[:])
            osc = cp.tile([ng, F], f32)
            nc.scalar.activation(out=osc[:], in_=psum_o[:], func=AF.Copy,
                                 scale=rcnt[:])
            ofin = cp.tile([ng, F], f32)
            nc.vector.tensor_add(out=ofin[:], in0=osc[:], in1=bl_sb[:])
            nc.sync.dma_start(out=out_p[:], in_=ofin[:])

    nc.compile()
    split_multi_waits(nc)
    return nc


def _run(inputs, trace=False, n_cores=CORES, bpc=BPC):
    import ml_dtypes
    x = np.asarray(inputs["x"], dtype=np.float32)
    edge_index = np.asarray(inputs["edge_index"])
    batch = np.asarray(inputs["batch"])
    ng = NG
    pp = _prep(x, edge_index, batch, n_cores=n_cores, bpc=bpc, ng=ng)

    iota8 = np.tile(np.arange(P, dtype=np.float32), (P, CPI)).astype(
        ml_dtypes.bfloat16).reshape(P, CPI * P)
    ident = np.eye(P, dtype=np.float32).astype(ml_dtypes.bfloat16)
    iotang = np.tile(np.arange(NG, dtype=np.float32), (P, 1))
    w1 = np.asarray(inputs["W1"], dtype=np.float32).astype(ml_dtypes.bfloat16)
    w2 = np.asarray(inputs["W2"], dtype=np.float32).astype(ml_dtypes.bfloat16)
    wl = np.asarray(inputs["Wl"], dtype=np.float32)
    b1bc = np.tile(np.asarray(inputs["b1"], dtype=np.float32), (P, 1))
    b2bc = np.tile(np.asarray(inputs["b2"], dtype=np.float32), (P, 1))
    blbc = np.tile(np.asarray(inputs["bl"], dtype=np.float32), (NG, 1))

    nc = _build(pp["meta1"], pp["meta2"], pp["vpad"], pp["bounds"],
                n_cores=n_cores, bpc=bpc, ng=ng)
    bf = ml_dtypes.bfloat16
    in_maps = []
    for c in range(n_cores):
        in_maps.append({
            "xt": pp["xt"],
            "xt_own": pp["xt"][c * bpc * P:(c + 1) * bpc * P],
            "idx1": pp["idx16_1"][c],
            "idx2": pp["idx16_2"][c],
            "dr1": pp["dstrel_1"][c].astype(bf),
            "dr2": pp["dstrel_2"][c].astype(bf),
            "dinvb": pp["dinvb"][c],
            "batchp": pp["batchp"][c],
            "cnt": pp["cnt"],
            "iota8": iota8,
            "ident": ident,
            "iotang": iotang,
            "W1": w1, "W2": w2, "Wl": wl,
            "b1bc": b1bc, "b2bc": b2bc, "blbc": blbc,
        })
    res = run_bass_kernel_spmd(nc, in_maps, list(range(n_cores)), trace=trace)
    return res.results[0]["out"], res.exec_time_ns


def kernel(**inputs) -> np.ndarray:
    out, _ = _run(inputs)
    return out


# revision 3
# speedup vs baseline: 1.0053x; 1.0053x over previous
"""GCN graph-embedding kernel v2 for 8 Trainium2 NeuronCores (Bass/Tile).

Strategy (dst-node sharding, dma_gather edition):
  - Nodes permuted + bin-packed into 128-node blocks balanced by in-degree,
    49 blocks/core. Per-block regular-edge lists are split into two int16
    index windows (lo = table rows [0, 32768), hi = rows [vpad-32768, vpad)),
    sorted by source row, and packed column-major into 128-edge chunks.
  - Source rows are fetched with InstDMAGatherAnt (SWDGE Q7 gather):
    1024 indices per instruction (HW limit), bf16 256B rows. The per-
    instruction descriptor-gen cost is ~1.5us + 0.34ns/row, ~6x cheaper per
    edge than per-128-row indirect DMAs.
  - Normalization is folded into the data: the gather table holds
    x~ = dinv[n] * x[n] (and between layers h~1 = dinv * relu(...)), so the
    aggregation is a plain masked sum; the remaining dinv[dst] factor is a
    per-partition activation scale after the W matmul.
  - Per chunk, a selector matrix sel[e, slot] = (iota == dstrel) routes edges
    to their dst slot on the TensorEngine, accumulating agg^T in PSUM.
    Selectors for a whole gather instruction are built in one DVE op via a
    stride-0 broadcast of the dstrel columns.
  - Self-loops skip the gather: the core's own (prescaled) block arrives via
    one strided DMA and joins the PSUM accumulation as an identity matmul.
  - Between layers each core's h~1 slice is exchanged with a chunked
    AllGather (chunk-major h table, pid2 addressing) exactly as in v1.
  - Global mean-pool is fused into layer 2 as a one-hot matmul accumulated in
    PSUM; partials are combined with a small AllReduce and the tiny linear
    head runs redundantly on every core.
"""
import numpy as np

import concourse.bass as bass
import concourse.bacc as bacc
import concourse.mybir as mybir
import concourse.tile as tile
from concourse.bass_utils import run_bass_kernel_spmd

F = 128          # feature width (all layers)
P = 128          # partitions / block size
CORES = 8
BPC = 49         # blocks per core
NG = 64          # number of graphs
WIN = 32768      # int16 index window size
CPI = 8          # max chunks per gather instruction (1024 idx HW cap)


def split_multi_waits(nc, max_waits: int = 1) -> int:
    n_split = 0
    f = nc.cur_f
    for bb in f.blocks:
        new_insts = []
        for inst in bb.instructions:
            si = inst.sync_info
            if si is not None and len(si.on_wait) > max_waits:
                waits = list(si.on_wait)
                extra, keep = waits[:-max_waits], waits[-max_waits:]
                for w in extra:
                    nop = mybir.InstNoOp(
                        name=nc.get_next_instruction_name(),
                        sync_info=mybir.SyncInfo(on_wait=[w], on_update=[]),
                        bass_nofuse=True,
                        engine=inst.engine,
                        ins=[],
                        outs=[],
                    )
                    nc.register_instruction(nop, overwrite=True)
                    new_insts.append(nop)
                inst.sync_info = mybir.SyncInfo(
                    on_wait=keep, on_update=list(si.on_update)
                )
                n_split += 1
            new_insts.append(inst)
        bb.instructions = new_insts
    return n_split


def _pack_layer(src_idx, e_dst_block, e_dst_slot, nblocks, n_cores, bpc):
    """Pack one layer's regular edges into lo/hi 128-edge chunk grids.

    src_idx: per-edge source row in the layer's gather table (pid or pid2).
    Returns per-core idx16 tables, dstrel tables, instruction metas and
    chunk-location maps (shared across cores: the shapes are uniform).
    """
    vpad = nblocks * P
    hi_base = vpad - WIN

    eorder = np.argsort(e_dst_block, kind="stable")
    es_idx = src_idx[eorder]
    es_slot = e_dst_slot[eorder]
    eb = np.bincount(e_dst_block, minlength=nblocks)
    eb_cum = np.concatenate([[0], np.cumsum(eb)])

    K = int(np.ceil(eb.max() / P))
    K_lo = (K + 1) // 2
    K_hi = K - K_lo
    cap_lo, cap_hi = K_lo * P, K_hi * P

    # per-block lo/hi assignment (balanced via the overlap window)
    blk_lo = []   # per block: (idx_local int16 [cap_lo], dstrel [cap_lo])
    blk_hi = []
    for b in range(nblocks):
        bi = es_idx[eb_cum[b]:eb_cum[b + 1]]
        bs = es_slot[eb_cum[b]:eb_cum[b + 1]]
        n = len(bi)
        forced_lo = bi < hi_base
        forced_hi = bi >= WIN
        mid = ~forced_lo & ~forced_hi
        n_flo, n_mid = int(forced_lo.sum()), int(mid.sum())
        assert n_flo <= cap_lo and int(forced_hi.sum()) <= cap_hi, (
            b, n_flo, int(forced_hi.sum()))
        # fill lo toward n/2, clamped by capacity on both sides
        want_lo = max(n_flo, min(n_flo + n_mid, (n + 1) // 2, cap_lo, n - 0))
        if n - want_lo > cap_hi:
            want_lo = n - cap_hi
        assert n_flo <= want_lo <= n_flo + n_mid and want_lo <= cap_lo
        take_mid = want_lo - n_flo
        lo_mask = forced_lo.copy()
        mid_pos = np.where(mid)[0]
        lo_mask[mid_pos[:take_mid]] = True
        for mask, cap, base, out in (
            (lo_mask, cap_lo, 0, blk_lo),
            (~lo_mask, cap_hi, hi_base, blk_hi),
        ):
            ii = bi[mask]
            ss = bs[mask]
            o = np.argsort(ii, kind="stable")
            ii, ss = ii[o] - base, ss[o]
            m = len(ii)
            assert m <= cap
            idx = np.zeros(cap, dtype=np.int16)
            rel = np.full(cap, -1.0, dtype=np.float32)
            idx[:m] = ii.astype(np.int16)
            rel[:m] = ss.astype(np.float32)
            out.append((idx, rel))

    # per-core chunk streams + instruction packing
    n_lo_chunks = bpc * K_lo
    n_hi_chunks = bpc * K_hi
    lo_n_instr = int(np.ceil(n_lo_chunks / CPI))
    hi_n_instr = int(np.ceil(n_hi_chunks / CPI))

    # issue order: interleave lo/hi instructions
    issue = []   # (window, start_chunk, n_chunks)
    li = hi = 0
    while li < lo_n_instr or hi < hi_n_instr:
        if li < lo_n_instr:
            s = li * CPI
            issue.append(("lo", s, min(CPI, n_lo_chunks - s)))
            li += 1
        if hi < hi_n_instr:
            s = hi * CPI
            issue.append(("hi", s, min(CPI, n_hi_chunks - s)))
            hi += 1

    # global column position of each instruction's first chunk
    instr_col0 = np.cumsum([0] + [m[2] for m in issue])[:-1]
    total_chunks = n_lo_chunks + n_hi_chunks

    # map (window, global window-chunk id) -> (instr index, pos)
    lo_map = {}
    hi_map = {}
    for i, (win, s, nch) in enumerate(issue):
        for p_ in range(nch):
            (lo_map if win == "lo" else hi_map)[s + p_] = (i, p_)

    # chunk_loc[b] = ordered (instr, pos) covering block b's K chunks
    chunk_loc = []
    need_instr = []
    for lb in range(bpc):
        locs = [lo_map[lb * K_lo + k] for k in range(K_lo)]
        locs += [hi_map[lb * K_hi + k] for k in range(K_hi)]
        chunk_loc.append(locs)
        need_instr.append(max(i for i, _ in locs))

    # per-core tables in issue order
    idx16 = np.zeros((n_cores, P, total_chunks * CPI), dtype=np.int16)
    dstrel = np.full((n_cores, P, total_chunks), -1.0, dtype=np.float32)
    for c in range(n_cores):
        col = 0
        for win, s, nch in issue:
            blk = blk_lo if win == "lo" else blk_hi
            kk = K_lo if win == "lo" else K_hi
            for p_ in range(nch):
                wc = s + p_        # window chunk id
                lb, k = divmod(wc, kk)
                idx, rel = blk[c * bpc + lb]
                ch_idx = idx[k * P:(k + 1) * P]
                ch_rel = rel[k * P:(k + 1) * P]
                # interleaved idx layout: idx j at (j%16, col*8 + j//16)
                lay = ch_idx.reshape(CPI, 16).T     # [16, 8]
                for g in range(CPI):
                    idx16[c, g * 16:(g + 1) * 16,
                          (col + p_) * CPI:(col + p_ + 1) * CPI] = lay
                dstrel[c, :, col + p_] = ch_rel
            col += nch

    meta = dict(issue=issue, instr_col0=instr_col0.tolist(),
                chunk_loc=chunk_loc, need_instr=need_instr,
                total_chunks=total_chunks, K=K)
    return idx16, dstrel, meta


def _prep(x, edge_index, batch, n_cores=CORES, bpc=BPC, ng=NG):
    """Host-side preprocessing: node permutation, degree norms, bf16 table,
    per-layer gather-chunk packing. Index manipulation + one prescale pass."""
    import heapq
    import ml_dtypes

    n = x.shape[0]
    src = np.asarray(edge_index[0], dtype=np.int64)
    dst = np.asarray(edge_index[1], dtype=np.int64)
    w_reg = np.bincount(dst, minlength=n).astype(np.int64)
    deg = w_reg + 1  # incl self-loop (PyG GCNConv norm)
    dinv = 1.0 / np.sqrt(deg.astype(np.float64))

    nblocks = n_cores * bpc
    cap = np.full(nblocks, P, dtype=np.int64)
    assert cap.sum() >= n

    order = np.argsort(-w_reg, kind="stable")
    heap = [(0, b) for b in range(nblocks)]
    heapq.heapify(heap)
    fill = np.zeros(nblocks, dtype=np.int64)
    node_block = np.empty(n, dtype=np.int64)
    node_slot = np.empty(n, dtype=np.int64)
    for nd in order:
        while True:
            load, b = heapq.heappop(heap)
            if fill[b] < cap[b]:
                break
        node_block[nd] = b
        node_slot[nd] = fill[b]
        fill[b] += 1
        if fill[b] < cap[b]:
            heapq.heappush(heap, (load + int(w_reg[nd]), b))

    vpad = nblocks * P
    pid = node_block * P + node_slot

    # chunk-major h-table layout for the chunked AllGather (layer 2)
    nchunks = min(6, bpc)
    last = max(1, bpc // 24)
    rest = bpc - last
    bounds = [round(i * rest / (nchunks - 1)) for i in range(nchunks)] + [bpc]
    gstart = [n_cores * P * b for b in bounds]
    lb_all = node_block % bpc
    c_all = node_block // bpc
    ch_all = np.searchsorted(bounds, lb_all, side="right") - 1
    rows_ch = np.array([(bounds[j + 1] - bounds[j]) * P for j in range(nchunks)])
    pid2 = (np.array(gstart)[ch_all] + c_all * rows_ch[ch_all]
            + (lb_all - np.array(bounds)[ch_all]) * P + node_slot)

    # prescaled bf16 gather table for layer 1
    xt = np.zeros((vpad, F), dtype=ml_dtypes.bfloat16)
    xt[pid] = (np.asarray(x, dtype=np.float64)
               * dinv[:, None]).astype(ml_dtypes.bfloat16)

    # per-layer edge packing (regular edges only; self-loops via identity)
    e_dst_block = node_block[dst]
    e_dst_slot = node_slot[dst]
    idx16_1, dstrel_1, meta1 = _pack_layer(
        pid[src], e_dst_block, e_dst_slot, nblocks, n_cores, bpc)
    idx16_2, dstrel_2, meta2 = _pack_layer(
        pid2[src], e_dst_block, e_dst_slot, nblocks, n_cores, bpc)

    # per-core per-slot tables
    dinvb = np.ones((n_cores, P, bpc), dtype=np.float32)
    batchp = np.full((n_cores, P, bpc), -1.0, dtype=np.float32)
    bt = np.asarray(batch, dtype=np.int64)
    for c in range(n_cores):
        mask = (node_block >= c * bpc) & (node_block < (c + 1) * bpc)
        sl = node_slot[mask]
        nb = node_block[mask] - c * bpc
        dinvb[c, sl, nb] = dinv[mask].astype(np.float32)
        batchp[c, sl, nb] = bt[mask].astype(np.float32)

    cnt = np.bincount(bt, minlength=ng).astype(np.float32)[:, None]
    return dict(xt=xt, idx16_1=idx16_1, dstrel_1=dstrel_1, meta1=meta1,
                idx16_2=idx16_2, dstrel_2=dstrel_2, meta2=meta2,
                dinvb=dinvb, batchp=batchp, cnt=cnt,
                vpad=vpad, bounds=bounds)


def _build(meta1, meta2, vpad, bounds, n_cores=CORES, bpc=BPC, ng=NG,
           debug=False):
    f32 = mybir.dt.float32
    bf16 = mybir.dt.bfloat16
    AF = mybir.ActivationFunctionType
    nc = bacc.Bacc(None, target_bir_lowering=False, debug=debug)

    tc1 = meta1["total_chunks"]
    tc2 = meta2["total_chunks"]
    hi_base = vpad - WIN
    slice_rows = bpc * P

    xt_p = nc.declare_dram_parameter("xt", [vpad, F], bf16, isOutput=False)
    xtown_p = nc.declare_dram_parameter("xt_own", [bpc * P, F], bf16,
                                        isOutput=False)
    idx1_p = nc.declare_dram_parameter("idx1", [P, tc1 * CPI], mybir.dt.int16,
                                       isOutput=False)
    idx2_p = nc.declare_dram_parameter("idx2", [P, tc2 * CPI], mybir.dt.int16,
                                       isOutput=False)
    dr1_p = nc.declare_dram_parameter("dr1", [P, tc1], bf16, isOutput=False)
    dr2_p = nc.declare_dram_parameter("dr2", [P, tc2], bf16, isOutput=False)
    dinvb_p = nc.declare_dram_parameter("dinvb", [P, bpc], f32, isOutput=False)
    batch_p = nc.declare_dram_parameter("batchp", [P, bpc], f32, isOutput=False)
    cnt_p = nc.declare_dram_parameter("cnt", [ng, 1], f32, isOutput=False)
    iota8_p = nc.declare_dram_parameter("iota8", [P, CPI * P], bf16,
                                        isOutput=False)
    ident_p = nc.declare_dram_parameter("ident", [P, P], bf16, isOutput=False)
    iotang_p = nc.declare_dram_parameter("iotang", [P, ng], f32, isOutput=False)
    w1_p = nc.declare_dram_parameter("W1", [F, F], bf16, isOutput=False)
    w2_p = nc.declare_dram_parameter("W2", [F, F], bf16, isOutput=False)
    wl_p = nc.declare_dram_parameter("Wl", [F, F], f32, isOutput=False)
    b1_p = nc.declare_dram_parameter("b1bc", [P, F], f32, isOutput=False)
    b2_p = nc.declare_dram_parameter("b2bc", [P, F], f32, isOutput=False)
    bl_p = nc.declare_dram_parameter("blbc", [ng, F], f32, isOutput=False)
    out_p = nc.declare_dram_parameter("out", [ng, F], f32, isOutput=True)

    with tile.TileContext(nc) as tc:
        with (
            tc.tile_pool(name="dram", bufs=1, space="DRAM") as dram,
            tc.tile_pool(name="const", bufs=1) as cp,
            tc.tile_pool(name="gp", bufs=6) as gp,
            tc.tile_pool(name="sp", bufs=6) as spool,
            tc.tile_pool(name="bp", bufs=4) as bpool,
            tc.tile_pool(name="ps", bufs=2, space="PSUM") as psp,
            tc.tile_pool(name="psagg", bufs=3, space="PSUM") as psagg,
            tc.tile_pool(name="psacc", bufs=1, space="PSUM") as psacc,
        ):
            ag_in = dram.tile([slice_rows, F], bf16)
            h_tab = dram.tile([vpad, F], bf16)
            ar_in = dram.tile([F, ng], f32)
            ar_out = dram.tile([F, ng], f32)

            idx1_sb = cp.tile([P, tc1 * CPI], mybir.dt.int16)
            nc.sync.dma_start(out=idx1_sb[:], in_=idx1_p[:])
            idx2_sb = cp.tile([P, tc2 * CPI], mybir.dt.int16)
            nc.sync.dma_start(out=idx2_sb[:], in_=idx2_p[:])
            dr1_sb = cp.tile([P, tc1], bf16)
            nc.sync.dma_start(out=dr1_sb[:], in_=dr1_p[:])
            dr2_sb = cp.tile([P, tc2], bf16)
            nc.sync.dma_start(out=dr2_sb[:], in_=dr2_p[:])
            dinvb_sb = cp.tile([P, bpc], f32)
            nc.sync.dma_start(out=dinvb_sb[:], in_=dinvb_p[:])
            batch_sb = cp.tile([P, bpc], f32)
            nc.sync.dma_start(out=batch_sb[:], in_=batch_p[:])
            iota8_sb = cp.tile([P, CPI * P], bf16)
            nc.sync.dma_start(out=iota8_sb[:], in_=iota8_p[:])
            ident_sb = cp.tile([P, P], bf16)
            nc.sync.dma_start(out=ident_sb[:], in_=ident_p[:])
            iotang_sb = cp.tile([P, ng], f32)
            nc.sync.dma_start(out=iotang_sb[:], in_=iotang_p[:])
            w1_sb = cp.tile([F, F], bf16)
            nc.sync.dma_start(out=w1_sb[:], in_=w1_p[:])
            w2_sb = cp.tile([F, F], bf16)
            nc.sync.dma_start(out=w2_sb[:], in_=w2_p[:])
            wl_sb = cp.tile([F, F], f32)
            nc.sync.dma_start(out=wl_sb[:], in_=wl_p[:])
            b1_sb = cp.tile([P, F], f32)
            nc.sync.dma_start(out=b1_sb[:], in_=b1_p[:])
            b2_sb = cp.tile([P, F], f32)
            nc.sync.dma_start(out=b2_sb[:], in_=b2_p[:])
            bl_sb = cp.tile([ng, F], f32)
            nc.sync.dma_start(out=bl_sb[:], in_=bl_p[:])
            cnt_sb = cp.tile([ng, 1], f32)
            nc.sync.dma_start(out=cnt_sb[:], in_=cnt_p[:])

            pool_acc = psacc.tile([F, ng], f32)

            def layer(meta, src_tab, self_src, idx_sb, dr_sb, w_sb, bbc_sb,
                      is_last, post_block=None):
                issue = meta["issue"]
                col0s = meta["instr_col0"]
                chunk_loc = meta["chunk_loc"]
                need_instr = meta["need_instr"]
                K = meta["K"]

                selfb = cp.tile([P, bpc * F], bf16, tag="selfb")
                nc.sync.dma_start(
                    out=selfb[:].rearrange("p (b f) -> p b f", f=F),
                    in_=self_src.rearrange("(b p) f -> p b f", p=P),
                )
                gtiles = [None] * len(issue)
                seltiles = [None] * len(issue)

                def emit_gather(i):
                    win, _s, nch = issue[i]
                    c0 = col0s[i]
                    src_win = (src_tab[0:WIN, :] if win == "lo"
                               else src_tab[hi_base:vpad, :])
                    gt = gp.tile([P, nch * F], bf16, tag="g")
                    nc.gpsimd.dma_gather(
                        gt[:].rearrange("p (c f) -> p c f", f=F),
                        src_win,
                        idx_sb[:, c0 * CPI:(c0 + nch) * CPI],
                        nch * P,
                        nch * P,
                        F,
                    )
                    st = spool.tile([P, nch * P], bf16, tag="sel")
                    nc.vector.tensor_tensor(
                        out=st[:],
                        in0=iota8_sb[:, :nch * P],
                        in1=dr_sb[:, c0:c0 + nch].to_broadcast([P, nch, P]),
                        op=mybir.AluOpType.is_equal,
                    )
                    gtiles[i] = gt
                    seltiles[i] = st

                emitted = 0
                for b in range(bpc):
                    while emitted <= need_instr[b]:
                        emit_gather(emitted)
                        emitted += 1
                    psum_agg = psagg.tile([F, P], f32, tag="agg")
                    # self-loop contribution: x~[d] via identity routing
                    nc.tensor.matmul(
                        out=psum_agg[:], lhsT=selfb[:, b * F:(b + 1) * F],
                        rhs=ident_sb[:], start=True, stop=False,
                    )
                    for j, (i, pos) in enumerate(chunk_loc[b]):
                        nc.tensor.matmul(
                            out=psum_agg[:],
                            lhsT=gtiles[i][:, pos * F:(pos + 1) * F],
                            rhs=seltiles[i][:, pos * P:(pos + 1) * P],
                            start=False, stop=(j == K - 1),
                        )
                    aggT = bpool.tile([F, P], bf16, tag="aggT")
                    nc.vector.tensor_copy(out=aggT[:], in_=psum_agg[:])
                    psum_h = psp.tile([P, F], f32, tag="h")
                    nc.tensor.matmul(out=psum_h[:], lhsT=aggT[:], rhs=w_sb[:],
                                     start=True, stop=True)
                    t1 = bpool.tile([P, F], f32, tag="t1")
                    nc.scalar.activation(out=t1[:], in_=psum_h[:], func=AF.Copy,
                                         scale=dinvb_sb[:, b:b + 1])
                    hb = bpool.tile([P, F], f32, tag="hb")
                    nc.vector.tensor_add(out=hb[:], in0=t1[:], in1=bbc_sb[:])
                    if not is_last:
                        # h~1 = dinv * relu(hb) = relu(dinv * hb); dinv > 0
                        hrt = bpool.tile([P, F], bf16, tag="hrt")
                        nc.scalar.activation(out=hrt[:], in_=hb[:], func=AF.Relu,
                                             scale=dinvb_sb[:, b:b + 1])
                        nc.sync.dma_start(
                            out=ag_in[b * P:(b + 1) * P, :], in_=hrt[:])
                    else:
                        hr = bpool.tile([P, F], bf16, tag="hr")
                        nc.scalar.activation(out=hr[:], in_=hb[:], func=AF.Relu)
                        gbh = bpool.tile([P, ng], bf16, tag="Gh")
                        nc.vector.tensor_tensor(
                            out=gbh[:],
                            in0=batch_sb[:, b:b + 1].to_broadcast([P, ng]),
                            in1=iotang_sb[:],
                            op=mybir.AluOpType.is_equal,
                        )
                        nc.tensor.matmul(out=pool_acc[:], lhsT=hr[:],
                                         rhs=gbh[:],
                                         start=(b == 0), stop=(b == bpc - 1))
                    if post_block is not None:
                        post_block(b)

            nchunks = len(bounds) - 1

            def post_block(b):
                for j in range(nchunks):
                    if b == bounds[j + 1] - 1:
                        rows = (bounds[j + 1] - bounds[j]) * P
                        gs = n_cores * P * bounds[j]
                        nc.gpsimd.collective_compute(
                            "AllGather",
                            mybir.AluOpType.bypass,
                            replica_groups=[list(range(n_cores))],
                            ins=[ag_in[bounds[j] * P:bounds[j + 1] * P, :]],
                            outs=[h_tab[gs:gs + n_cores * rows, :]],
                        )

            layer(meta1, xt_p, xtown_p[:], idx1_sb, dr1_sb, w1_sb, b1_sb,
                  is_last=False, post_block=post_block)
            layer(meta2, h_tab, ag_in[:], idx2_sb, dr2_sb, w2_sb, b2_sb,
                  is_last=True)

            poolT_sb = cp.tile([F, ng], f32)
            nc.vector.tensor_copy(out=poolT_sb[:], in_=pool_acc[:])
            nc.gpsimd.dma_start(out=ar_in[:], in_=poolT_sb[:])
            nc.gpsimd.collective_compute(
                "AllReduce",
                mybir.AluOpType.add,
                replica_groups=[list(range(n_cores))],
                ins=[ar_in.opt()],
                outs=[ar_out.opt()],
            )
            poolT_ar = cp.tile([F, ng], f32)
            nc.gpsimd.dma_start(out=poolT_ar[:], in_=ar_out[:])

            psum_o = psp.tile([ng, F], f32, tag="o")
            nc.tensor.matmul(out=psum_o[:], lhsT=poolT_ar[:], rhs=wl_sb[:],
                             start=True, stop=True)
            cmax = cp.tile([ng, 1], f32)
            nc.vector.tensor_scalar(out=cmax[:], in0=cnt_sb[:], scalar1=1.0,
                                    scalar2=None, op0=mybir.AluOpType.max)
            rcnt = cp.tile([ng, 1], f32)
            nc.vector.reciprocal(out=rcnt[:], in_=cmax[:])
            osc = cp.tile([ng, F], f32)
            nc.scalar.activation(out=osc[:], in_=psum_o[:], func=AF.Copy,
                                 scale=rcnt[:])
            ofin = cp.tile([ng, F], f32)
            nc.vector.tensor_add(out=ofin[:], in0=osc[:], in1=bl_sb[:])
            nc.sync.dma_start(out=out_p[:], in_=ofin[:])

    nc.compile()
    split_multi_waits(nc)
    return nc


def _run(inputs, trace=False, n_cores=CORES, bpc=BPC):
    import ml_dtypes
    x = np.asarray(inputs["x"], dtype=np.float32)
    edge_index = np.asarray(inputs["edge_index"])
    batch = np.asarray(inputs["batch"])
    ng = NG
    pp = _prep(x, edge_index, batch, n_cores=n_cores, bpc=bpc, ng=ng)

    iota8 = np.tile(np.arange(P, dtype=np.float32), (P, CPI)).astype(
        ml_dtypes.bfloat16).reshape(P, CPI * P)
    ident = np.eye(P, dtype=np.float32).astype(ml_dtypes.bfloat16)
    iotang = np.tile(np.arange(NG, dtype=np.float32), (P, 1))
    w1 = np.asarray(inputs["W1"], dtype=np.float32).astype(ml_dtypes.bfloat16)
    w2 = np.asarray(inputs["W2"], dtype=np.float32).astype(ml_dtypes.bfloat16)
    wl = np.asarray(inputs["Wl"], dtype=np.float32)
    b1bc = np.tile(np.asarray(inputs["b1"], dtype=np.float32), (P, 1))
    b2bc = np.tile(np.asarray(inputs["b2"], dtype=np.float32), (P, 1))
    blbc = np.tile(np.asarray(inputs["bl"], dtype=np.float32), (NG, 1))

    nc = _build(pp["meta1"], pp["meta2"], pp["vpad"], pp["bounds"],
                n_cores=n_cores, bpc=bpc, ng=ng)
    bf = ml_dtypes.bfloat16
    in_maps = []
    for c in range(n_cores):
        in_maps.append({
            "xt": pp["xt"],
            "xt_own": pp["xt"][c * bpc * P:(c + 1) * bpc * P],
            "idx1": pp["idx16_1"][c],
            "idx2": pp["idx16_2"][c],
            "dr1": pp["dstrel_1"][c].astype(bf),
            "dr2": pp["dstrel_2"][c].astype(bf),
            "dinvb": pp["dinvb"][c],
            "batchp": pp["batchp"][c],
            "cnt": pp["cnt"],
            "iota8": iota8,
            "ident": ident,
            "iotang": iotang,
            "W1": w1, "W2": w2, "Wl": wl,
            "b1bc": b1bc, "b2bc": b2bc, "blbc": blbc,
        })
    res = run_bass_kernel_spmd(nc, in_maps, list(range(n_cores)), trace=trace)
    return res.results[0]["out"], res.exec_time_ns


def kernel(**inputs) -> np.ndarray:
    out, _ = _run(inputs)
    return out
